# revision 28
# baseline (speedup 1.0000x reference)
"""HRALinear forward on 8 Trainium2 NeuronCores (Bass/Tile), fp8 DoubleRow.

Math (compact-WY form of the sequential Householder scan):
  u_i = hra_u[:, i] / ||hra_u[:, i]||
  H_0 H_1 ... H_{r-1} = I - U T U^T          (T upper triangular, T_ii = 2)
  out = X W^T + (X Uraw) S' (W Uraw)^T + 1 x bias^T
  with S' = -D T^T D, D = diag(1/||u_i||)    (S' is 8x8, host-computed)

The rank-8 correction (X Uraw) S' (W Uraw)^T is only ~500 MFLOP, so it is
computed on the host in fp32 and folded together with the bias into a
bf16 table corrb = corr + bias that VectorE adds during PSUM eviction.
The device program is then a pure GEMM: out = X @ W^T (+corrb).

Precision: operands are fp8-e4m3 so the PE runs in DoubleRow perf mode
(2 k-tiles per instruction; measured 157 TF/s/core, 2x bf16).  A single
e4m3 pass has max rel err 2.4e-2 (> the 2e-2 gate), so a partial second
pass refines W: Wlo = e4m3(32W - e4m3(32W)) is accumulated for the first
B=12 of 16 k-pairs, which lands at 1.89e-2 (host-sim matches HW bit-wise
for this pipeline; B=11 is 1.96e-2 - too close to the gate).  W is
carried as 32*W (avoids e4m3 denormals); eviction computes
out = psum*(1/32) + corrb in one VectorE scalar_tensor_tensor and writes
bf16 for the output DMA.

Per-core PE work: 32 o-tiles x 2 m-blocks x 28 DoubleRow matmuls at
~224 ns each, plus ~10 us of DMA-paced startup (panels are split A/B and
x is streamed per k-pair so the first matmuls begin ~2 us in).
Sharding: data-parallel over the 8192 batch*seq rows (1024 rows/core);
weights replicated.
"""

import os
import sys
from contextlib import ExitStack

os.environ.setdefault("MYCRO_LOCAL_CACHE", "1")
for _p in ("/opt/trn_rl_repo",):
    if os.path.isdir(_p) and _p not in sys.path:
        sys.path.insert(0, _p)

import ml_dtypes
import numpy as np

import concourse.bacc as bacc
import concourse.mybir as mybir
import concourse.tile as tile
from concourse.bass_utils import run_bass_kernel_spmd

P = 128          # partitions
N_CORES = 8
R = 8
B_WFIX = 12      # k-pairs (of 16) covered by the Wlo refinement pass
ASPLIT = 8       # panel pairs in the A (early) chunk
LDW_HOIST = False  # walrus ignores ldweights=False (still self-loads); keep off

F32 = mybir.dt.float32
F8 = mybir.dt.float8e4
BF16 = mybir.dt.bfloat16
NP_F8 = ml_dtypes.float8_e4m3
NP_BF16 = ml_dtypes.bfloat16

WSCALE = 32.0    # W uploaded as 32*W


def build_nc(M, N, K, B):
    """One-core SPMD program: outT[N,M] = X8 @ W-tilde^T / 32 + corrb.

    DRAM inputs (per core):
      xt    [P, KK2, 2, M]     X8^T k-pair-split (k = kk2*256 + i*128 + p)
      wt    [NT, P, WP, 2, P]  o-tile panels: 16 W8 pairs + B Wlo pairs
      corrb [NT, P, M] bf16    corrb[ot, p, m] = corr[m, ot*P+p] + bias[ot*P+p]
    DRAM output: outT [NT, P, M] bf16   (outT[ot, p, m] = out[m, ot*P+p])
    """
    KK2 = K // (2 * P)   # 16 k-pairs
    WP = KK2 + B         # w pairs incl. Wlo refinement
    NT = N // P
    MBW = min(512, M)   # 512 moving elements is the ISA max per matmul
    MB = M // MBW
    DR = mybir.MatmulPerfMode.DoubleRow

    nc = bacc.Bacc()
    xt = nc.dram_tensor("xt", [P, KK2, 2, M], F8, kind="ExternalInput")
    wt = nc.dram_tensor("wt", [NT, P, WP, 2, P], F8, kind="ExternalInput")
    corrb = nc.dram_tensor("corrb", [NT, P, M], BF16, kind="ExternalInput")
    outd = nc.dram_tensor("out", [NT, P, M], BF16, kind="ExternalOutput")

    with tile.TileContext(nc) as tc, ExitStack() as ctx:
        xpool = ctx.enter_context(tc.tile_pool(name="xpool", bufs=1))
        wpool = ctx.enter_context(tc.tile_pool(name="wpool", bufs=3))
        cpool = ctx.enter_context(tc.tile_pool(name="cpool", bufs=3))
        stage = ctx.enter_context(tc.tile_pool(name="stage", bufs=4))
        ps_out = ctx.enter_context(tc.tile_pool(name="ps_out", bufs=4, space="PSUM"))

        panels = {}
        corrs = {}

        def issue_panel(ot, split=False):
            wa = wpool.tile([P, 2 * ASPLIT, P], F8, tag="wpa", name=f"wpa{ot}")
            wb = wpool.tile([P, 2 * (WP - ASPLIT), P], F8, tag="wpb", name=f"wpb{ot}")
            nc.sync.dma_start(out=wa[:], in_=wt[ot, :, :ASPLIT, :, :])
            if not split:
                nc.sync.dma_start(out=wb[:], in_=wt[ot, :, ASPLIT:, :, :])
            cb = cpool.tile([P, M], BF16, tag="cb", name=f"cb{ot}")
            nc.sync.dma_start(out=cb[:], in_=corrb[ot])
            panels[ot] = (wa, wb)
            corrs[ot] = cb

        # startup: first panel's A chunk, then x pairs stream, B chunk between.
        # x pairs alternate between the sync and scalar DMA queues so the
        # upload runs at two-queue aggregate bandwidth.
        issue_panel(0, split=True)
        xs = []
        for j in range(KK2):
            xj = xpool.tile([P, 2, M], F8, name=f"x{j}")
            xq = nc.scalar if j % 2 else nc.sync
            xq.dma_start(out=xj[:], in_=xt[:, j, :, :])
            xs.append(xj)
            if j == 2:
                nc.sync.dma_start(
                    out=panels[0][1][:], in_=wt[0, :, ASPLIT:, :, :]
                )
        issue_panel(1)

        # per-group matmul schedule: (w-pair index, x tile index)
        sched = [(j, j) for j in range(KK2)] + [(KK2 + b, b) for b in range(B)]

        def wslice(wp, j):
            wa, wb = wp
            if j < ASPLIT:
                return wa[:, 2 * j : 2 * j + 2, :]
            j -= ASPLIT
            return wb[:, 2 * j : 2 * j + 2, :]

        for ot in range(NT):
            wp = panels.pop(ot)
            cb = corrs.pop(ot)
            if LDW_HOIST:
                # one explicit weight load per pair; the two m-blocks' matmuls
                # reuse the loaded stationary tile (ldweights=False)
                psos = [
                    ps_out.tile([P, MBW], F32, tag="ps", name=f"ps{ot}_{mb}")
                    for mb in range(MB)
                ]
                for n, (wj, xj) in enumerate(sched):
                    nc.tensor.ldweights(wslice(wp, wj), perf_mode=DR)
                    for mb in range(MB):
                        ms = slice(mb * MBW, (mb + 1) * MBW)
                        mm = nc.tensor.matmul(
                            psos[mb][:],
                            wslice(wp, wj),
                            xs[xj][:, :, ms],
                            start=(n == 0),
                            stop=(n == len(sched) - 1),
                            perf_mode=DR,
                        )
                        mm.ins.ldweights = False
                for mb in range(MB):
                    ms = slice(mb * MBW, (mb + 1) * MBW)
                    st = stage.tile([P, MBW], BF16, tag="st")
                    nc.vector.scalar_tensor_tensor(
                        st[:],
                        psos[mb][:],
                        1.0 / WSCALE,
                        cb[:, ms],
                        mybir.AluOpType.mult,
                        mybir.AluOpType.add,
                    )
                    nc.scalar.dma_start(out=outd[ot, :, ms], in_=st[:])
            else:
                for mb in range(MB):
                    ms = slice(mb * MBW, (mb + 1) * MBW)
                    pso = ps_out.tile([P, MBW], F32, tag="ps", name=f"ps{ot}_{mb}")
                    for n, (wj, xj) in enumerate(sched):
                        nc.tensor.matmul(
                            pso[:],
                            wslice(wp, wj),
                            xs[xj][:, :, ms],
                            start=(n == 0),
                            stop=(n == len(sched) - 1),
                            perf_mode=DR,
                        )
                    st = stage.tile([P, MBW], BF16, tag="st")
                    nc.vector.scalar_tensor_tensor(
                        st[:],
                        pso[:],
                        1.0 / WSCALE,
                        cb[:, ms],
                        mybir.AluOpType.mult,
                        mybir.AluOpType.add,
                    )
                    nc.sync.dma_start(out=outd[ot, :, ms], in_=st[:])
            if ot + 2 < NT:
                issue_panel(ot + 2)

    nc.compile()
    return nc


_NC_CACHE = {}


def get_nc(M, N, K, B):
    key = (M, N, K, B)
    if key not in _NC_CACHE:
        _NC_CACHE[key] = build_nc(M, N, K, B)
    return _NC_CACHE[key]


def compute_sprime(hra_u):
    """S' with out = X W^T + (X Uraw) S' (W Uraw)^T."""
    r = hra_u.shape[1]
    U = np.asarray(hra_u, dtype=np.float64)
    nrm = np.linalg.norm(U, axis=0)
    Uh = U / nrm
    G = Uh.T @ Uh
    T = np.zeros((r, r))
    for k in range(r):
        T[k, k] = 2.0
        if k:
            T[:k, k] = -2.0 * (T[:k, :k] @ G[:k, k])
    return -(T.T) / nrm[:, None] / nrm[None, :]


def kpair_split(a8, M, KK2):
    """[M, K] fp8 row-major -> [P, KK2, 2, M] with k = kk2*256 + i*128 + p."""
    return np.ascontiguousarray(a8.reshape(M, KK2, 2, P).transpose(3, 1, 2, 0))


def prepare(x, hra_u, base_weight, bias):
    x = np.asarray(x, dtype=np.float32)
    hra_u = np.asarray(hra_u, dtype=np.float32)
    W = np.asarray(base_weight, dtype=np.float32)
    bias = np.asarray(bias, dtype=np.float32)

    B_, S, K = x.shape
    N = W.shape[0]
    Mtot = B_ * S
    M = Mtot // N_CORES
    KK2 = K // (2 * P)
    NT = N // P

    X = x.reshape(Mtot, K)
    Sp = compute_sprime(hra_u)
    CS = (W.astype(np.float64) @ hra_u.astype(np.float64) @ Sp.T).astype(
        np.float32
    )                                                          # [N, R]
    Pm = X @ hra_u                                             # [Mtot, R]

    X8 = X.astype(NP_F8)
    W32 = WSCALE * W
    W8 = W32.astype(NP_F8)                                     # [N, K]
    Wlo8 = (W32 - W8.astype(np.float32)).astype(NP_F8)

    # wt panels: [NT, P, WP, 2, P] = [16 W8 pairs | B Wlo pairs]
    wmain = W8.reshape(NT, P, KK2, 2, P).transpose(0, 4, 2, 3, 1)
    wlo = Wlo8.reshape(NT, P, KK2, 2, P).transpose(0, 4, 2, 3, 1)[:, :, :B_WFIX]
    wt_host = np.ascontiguousarray(np.concatenate([wmain, wlo], axis=2))

    nc = get_nc(M, N, K, B_WFIX)

    in_maps = []
    for c in range(N_CORES):
        sl = slice(c * M, (c + 1) * M)
        xt_host = kpair_split(X8[sl], M, KK2)
        corrb = ((Pm[sl] @ CS.T) + bias).T.reshape(NT, P, M).astype(NP_BF16)
        in_maps.append(
            {"xt": xt_host, "wt": wt_host, "corrb": np.ascontiguousarray(corrb)}
        )
    return nc, in_maps, (B_, S, M, N)


def collect(res, meta):
    B_, S, M, N = meta
    shards = [
        np.asarray(r["out"]).reshape(N, M).T.astype(np.float32) for r in res
    ]
    out = np.concatenate(shards, axis=0)
    return np.ascontiguousarray(out.reshape(B_, S, N))


def kernel(x, hra_u, base_weight, bias):
    nc, in_maps, meta = prepare(x, hra_u, base_weight, bias)
    res = run_bass_kernel_spmd(nc, in_maps, core_ids=list(range(N_CORES))).results
    return collect(res, meta)


# revision 30
# speedup vs baseline: 1.0073x; 1.0073x over previous
"""HRALinear forward on 8 Trainium2 NeuronCores (Bass/Tile), fp8 DoubleRow.

Math (compact-WY form of the sequential Householder scan):
  u_i = hra_u[:, i] / ||hra_u[:, i]||
  H_0 H_1 ... H_{r-1} = I - U T U^T          (T upper triangular, T_ii = 2)
  out = X W^T + (X Uraw) S' (W Uraw)^T + 1 x bias^T
  with S' = -D T^T D, D = diag(1/||u_i||)    (S' is 8x8, host-computed)

The rank-8 correction (X Uraw) S' (W Uraw)^T is only ~500 MFLOP, so it is
computed on the host in fp32 and folded together with the bias into a
bf16 table corrb = corr + bias that VectorE adds during PSUM eviction.
The device program is then a pure GEMM: out = X @ W^T (+corrb).

Precision: operands are fp8-e4m3 so the PE runs in DoubleRow perf mode
(2 k-tiles per instruction; measured 157 TF/s/core, 2x bf16).  A single
e4m3 pass has max rel err 2.4e-2 (> the 2e-2 gate), so a partial second
pass refines W: Wlo = e4m3(32W - e4m3(32W)) is accumulated for the first
B=12 of 16 k-pairs, which lands at 1.89e-2 (host-sim matches HW bit-wise
for this pipeline; B=11 is 1.96e-2 - too close to the gate).  W is
carried as 32*W (avoids e4m3 denormals); eviction computes
out = psum*(1/32) + corrb in one VectorE scalar_tensor_tensor and writes
bf16 for the output DMA.

Per-core PE work: 32 o-tiles x 2 m-blocks x 28 DoubleRow matmuls at
~224 ns each, plus ~10 us of DMA-paced startup (panels are split A/B and
x is streamed per k-pair so the first matmuls begin ~2 us in).
Sharding: data-parallel over the 8192 batch*seq rows (1024 rows/core);
weights replicated.
"""

import os
import sys
from contextlib import ExitStack

os.environ.setdefault("MYCRO_LOCAL_CACHE", "1")
for _p in ("/opt/trn_rl_repo",):
    if os.path.isdir(_p) and _p not in sys.path:
        sys.path.insert(0, _p)

import ml_dtypes
import numpy as np

import concourse.bacc as bacc
import concourse.mybir as mybir
import concourse.tile as tile
from concourse.bass_utils import run_bass_kernel_spmd

P = 128          # partitions
N_CORES = 8
R = 8
B_WFIX = 12      # k-pairs (of 16) covered by the Wlo refinement pass
ASPLIT = 8       # panel pairs in the A (early) chunk
LDW_HOIST = False  # walrus ignores ldweights=False (still self-loads); keep off

F32 = mybir.dt.float32
F8 = mybir.dt.float8e4
BF16 = mybir.dt.bfloat16
NP_F8 = ml_dtypes.float8_e4m3
NP_BF16 = ml_dtypes.bfloat16

WSCALE = 32.0    # W uploaded as 32*W


def build_nc(M, N, K, B):
    """One-core SPMD program: outT[N,M] = X8 @ W-tilde^T / 32 + corrb.

    DRAM inputs (per core):
      xt    [P, KK2, 2, M]     X8^T k-pair-split (k = kk2*256 + i*128 + p)
      wt    [NT, P, WP, 2, P]  o-tile panels: 16 W8 pairs + B Wlo pairs
      corrb [NT, P, M] bf16    corrb[ot, p, m] = corr[m, ot*P+p] + bias[ot*P+p]
    DRAM output: outT [NT, P, M] bf16   (outT[ot, p, m] = out[m, ot*P+p])
    """
    KK2 = K // (2 * P)   # 16 k-pairs
    WP = KK2 + B         # w pairs incl. Wlo refinement
    NT = N // P
    MBW = min(512, M)   # 512 moving elements is the ISA max per matmul
    MB = M // MBW
    DR = mybir.MatmulPerfMode.DoubleRow

    nc = bacc.Bacc()
    xt = nc.dram_tensor("xt", [P, KK2, 2, M], F8, kind="ExternalInput")
    wt = nc.dram_tensor("wt", [NT, P, WP, 2, P], F8, kind="ExternalInput")
    corrb = nc.dram_tensor("corrb", [NT, P, M], BF16, kind="ExternalInput")
    outd = nc.dram_tensor("out", [NT, P, M], BF16, kind="ExternalOutput")

    with tile.TileContext(nc) as tc, ExitStack() as ctx:
        xpool = ctx.enter_context(tc.tile_pool(name="xpool", bufs=1))
        wpool = ctx.enter_context(tc.tile_pool(name="wpool", bufs=3))
        cpool = ctx.enter_context(tc.tile_pool(name="cpool", bufs=3))
        stage = ctx.enter_context(tc.tile_pool(name="stage", bufs=4))
        ps_out = ctx.enter_context(tc.tile_pool(name="ps_out", bufs=4, space="PSUM"))

        panels = {}
        corrs = {}

        def issue_panel(ot, split=False):
            wa = wpool.tile([P, 2 * ASPLIT, P], F8, tag="wpa", name=f"wpa{ot}")
            wb = wpool.tile([P, 2 * (WP - ASPLIT), P], F8, tag="wpb", name=f"wpb{ot}")
            nc.sync.dma_start(out=wa[:], in_=wt[ot, :, :ASPLIT, :, :])
            if not split:
                nc.sync.dma_start(out=wb[:], in_=wt[ot, :, ASPLIT:, :, :])
            cb = cpool.tile([P, M], BF16, tag="cb", name=f"cb{ot}")
            nc.sync.dma_start(out=cb[:], in_=corrb[ot])
            panels[ot] = (wa, wb)
            corrs[ot] = cb

        # startup: first panel's A chunk, then x pairs stream, B chunk between
        issue_panel(0, split=True)
        xs = []
        for j in range(KK2):
            xj = xpool.tile([P, 2, M], F8, name=f"x{j}")
            nc.sync.dma_start(out=xj[:], in_=xt[:, j, :, :])
            xs.append(xj)
            if j == 2:
                nc.sync.dma_start(
                    out=panels[0][1][:], in_=wt[0, :, ASPLIT:, :, :]
                )
        issue_panel(1)

        # per-group matmul schedule: (w-pair index, x tile index)
        sched = [(j, j) for j in range(KK2)] + [(KK2 + b, b) for b in range(B)]

        def wslice(wp, j):
            wa, wb = wp
            if j < ASPLIT:
                return wa[:, 2 * j : 2 * j + 2, :]
            j -= ASPLIT
            return wb[:, 2 * j : 2 * j + 2, :]

        for ot in range(NT):
            wp = panels.pop(ot)
            cb = corrs.pop(ot)
            if LDW_HOIST:
                # one explicit weight load per pair; the two m-blocks' matmuls
                # reuse the loaded stationary tile (ldweights=False)
                psos = [
                    ps_out.tile([P, MBW], F32, tag="ps", name=f"ps{ot}_{mb}")
                    for mb in range(MB)
                ]
                for n, (wj, xj) in enumerate(sched):
                    nc.tensor.ldweights(wslice(wp, wj), perf_mode=DR)
                    for mb in range(MB):
                        ms = slice(mb * MBW, (mb + 1) * MBW)
                        mm = nc.tensor.matmul(
                            psos[mb][:],
                            wslice(wp, wj),
                            xs[xj][:, :, ms],
                            start=(n == 0),
                            stop=(n == len(sched) - 1),
                            perf_mode=DR,
                        )
                        mm.ins.ldweights = False
                for mb in range(MB):
                    ms = slice(mb * MBW, (mb + 1) * MBW)
                    st = stage.tile([P, MBW], BF16, tag="st")
                    nc.vector.scalar_tensor_tensor(
                        st[:],
                        psos[mb][:],
                        1.0 / WSCALE,
                        cb[:, ms],
                        mybir.AluOpType.mult,
                        mybir.AluOpType.add,
                    )
                    nc.scalar.dma_start(out=outd[ot, :, ms], in_=st[:])
            else:
                for mb in range(MB):
                    ms = slice(mb * MBW, (mb + 1) * MBW)
                    pso = ps_out.tile([P, MBW], F32, tag="ps", name=f"ps{ot}_{mb}")
                    # pre-zero the bank on VectorE (runs ahead, overlapped) so
                    # no matmul pays the start=True zero-region setup
                    nc.vector.memset(pso[:], 0.0)
                    for n, (wj, xj) in enumerate(sched):
                        nc.tensor.matmul(
                            pso[:],
                            wslice(wp, wj),
                            xs[xj][:, :, ms],
                            start=False,
                            stop=(n == len(sched) - 1),
                            perf_mode=DR,
                            skip_group_check=True,
                        )
                    st = stage.tile([P, MBW], BF16, tag="st")
                    nc.vector.scalar_tensor_tensor(
                        st[:],
                        pso[:],
                        1.0 / WSCALE,
                        cb[:, ms],
                        mybir.AluOpType.mult,
                        mybir.AluOpType.add,
                    )
                    nc.sync.dma_start(out=outd[ot, :, ms], in_=st[:])
            if ot + 2 < NT:
                issue_panel(ot + 2)

    nc.compile()
    return nc


_NC_CACHE = {}


def get_nc(M, N, K, B):
    key = (M, N, K, B)
    if key not in _NC_CACHE:
        _NC_CACHE[key] = build_nc(M, N, K, B)
    return _NC_CACHE[key]


def compute_sprime(hra_u):
    """S' with out = X W^T + (X Uraw) S' (W Uraw)^T."""
    r = hra_u.shape[1]
    U = np.asarray(hra_u, dtype=np.float64)
    nrm = np.linalg.norm(U, axis=0)
    Uh = U / nrm
    G = Uh.T @ Uh
    T = np.zeros((r, r))
    for k in range(r):
        T[k, k] = 2.0
        if k:
            T[:k, k] = -2.0 * (T[:k, :k] @ G[:k, k])
    return -(T.T) / nrm[:, None] / nrm[None, :]


def kpair_split(a8, M, KK2):
    """[M, K] fp8 row-major -> [P, KK2, 2, M] with k = kk2*256 + i*128 + p."""
    return np.ascontiguousarray(a8.reshape(M, KK2, 2, P).transpose(3, 1, 2, 0))


def prepare(x, hra_u, base_weight, bias):
    x = np.asarray(x, dtype=np.float32)
    hra_u = np.asarray(hra_u, dtype=np.float32)
    W = np.asarray(base_weight, dtype=np.float32)
    bias = np.asarray(bias, dtype=np.float32)

    B_, S, K = x.shape
    N = W.shape[0]
    Mtot = B_ * S
    M = Mtot // N_CORES
    KK2 = K // (2 * P)
    NT = N // P

    X = x.reshape(Mtot, K)
    Sp = compute_sprime(hra_u)
    CS = (W.astype(np.float64) @ hra_u.astype(np.float64) @ Sp.T).astype(
        np.float32
    )                                                          # [N, R]
    Pm = X @ hra_u                                             # [Mtot, R]

    X8 = X.astype(NP_F8)
    W32 = WSCALE * W
    W8 = W32.astype(NP_F8)                                     # [N, K]
    Wlo8 = (W32 - W8.astype(np.float32)).astype(NP_F8)

    # wt panels: [NT, P, WP, 2, P] = [16 W8 pairs | B Wlo pairs]
    wmain = W8.reshape(NT, P, KK2, 2, P).transpose(0, 4, 2, 3, 1)
    wlo = Wlo8.reshape(NT, P, KK2, 2, P).transpose(0, 4, 2, 3, 1)[:, :, :B_WFIX]
    wt_host = np.ascontiguousarray(np.concatenate([wmain, wlo], axis=2))

    nc = get_nc(M, N, K, B_WFIX)

    in_maps = []
    for c in range(N_CORES):
        sl = slice(c * M, (c + 1) * M)
        xt_host = kpair_split(X8[sl], M, KK2)
        corrb = ((Pm[sl] @ CS.T) + bias).T.reshape(NT, P, M).astype(NP_BF16)
        in_maps.append(
            {"xt": xt_host, "wt": wt_host, "corrb": np.ascontiguousarray(corrb)}
        )
    return nc, in_maps, (B_, S, M, N)


def collect(res, meta):
    B_, S, M, N = meta
    shards = [
        np.asarray(r["out"]).reshape(N, M).T.astype(np.float32) for r in res
    ]
    out = np.concatenate(shards, axis=0)
    return np.ascontiguousarray(out.reshape(B_, S, N))


def kernel(x, hra_u, base_weight, bias):
    nc, in_maps, meta = prepare(x, hra_u, base_weight, bias)
    res = run_bass_kernel_spmd(nc, in_maps, core_ids=list(range(N_CORES))).results
    return collect(res, meta)
